# revision 4
# baseline (speedup 1.0000x reference)
"""Trainium2 Bass kernel for nn_EulerCEMinimal (mixed-head causal attention).

Reference model (B=2, N=2048, D=1024, H=16, dh=64):
  qkv = x @ w_qkv;  heads round-robin {nope, alibi, rope, xpos}
  rope/xpos heads: interleaved even/odd RoPE rotation of q,k
  alibi/xpos heads: additive decay bias -|q-k|/xi
  causal softmax;  out = (attn @ v) @ w_o

Sharding (8 cores): core r = (batch g=r//4) x (head-group c=r%4, heads 4c..4c+3).
Each core projects q/k transposed (feature-major) with an augmented K=66
contraction that adds the exact decay bias beta[k]-beta[q] inside the score
matmul; scores are computed transposed S^T[k,q] so no P-transposes are needed;
softmax normalization is deferred (row sums come free from a ones-column
appended to V) and folded into the attention output. Head outputs are
exchanged with an 8-core AllToAll (cross-batch shards zeroed via a data mask,
folded by an add on the receiver) so each core computes its own 512-row slice
of the output projection. All rank dependence lives in the input data; the
SPMD program is identical on every core.
"""

import sys
import types

sys.path.insert(0, "/opt/trn_rl_repo")

# Register the NTFF profile hook that the stub antenv package lacks (used only
# when tracing is requested, e.g. BASS_TRACE=1 or trace=True).
if "antenv.axon_hooks" not in sys.modules:
    _hooks = types.ModuleType("antenv.axon_hooks")
    _hooks._hook = None
    _hooks.set_axon_ntff_profile_hook = lambda h: setattr(_hooks, "_hook", h)
    _hooks.get_axon_ntff_profile_hook = lambda: _hooks._hook
    sys.modules["antenv.axon_hooks"] = _hooks
    try:
        from trn_agent_boot.trn_boot import _ntff_profile_via_ctypes

        _hooks.set_axon_ntff_profile_hook(
            _ntff_profile_via_ctypes("/opt/axon/libaxon_pjrt.so")
        )
    except Exception:
        pass

from contextlib import ExitStack

import numpy as np

import concourse.bass as bass
import concourse.tile as tile
from concourse import bacc, mybir
from concourse.bass_utils import run_bass_kernel_spmd
from concourse.masks import make_identity

F32 = mybir.dt.float32
B, N, D, H, DH = 2, 2048, 1024, 16, 64
NT = N // 128  # 16 query/key tiles
NCH = N // 512  # 4 projection column chunks
KT = D // 128  # 8 contraction tiles
ROPE_BASE = 10000.0
HEAD_TYPES = np.arange(H) % 4
PI_BITS = (HEAD_TYPES >> 1) & 1  # rotate (rope/xpos)
E_BITS = HEAD_TYPES & 1  # decay (alibi/xpos)

_CACHE = {}


def _build():
    nc = bacc.Bacc(None, target_bir_lowering=False, debug=False, num_devices=8)
    xt_d = nc.declare_dram_parameter("xt", [D, N], F32, isOutput=False)
    wqkv_d = nc.declare_dram_parameter("wqkv", [D, 768], F32, isOutput=False)
    wo_d = nc.declare_dram_parameter("wo", [D, D], F32, isOutput=False)
    cs_d = nc.declare_dram_parameter("cs", [128, N], F32, isOutput=False)
    beta_d = nc.declare_dram_parameter("beta", [10, N], F32, isOutput=False)
    gm_d = nc.declare_dram_parameter("gm", [128, 2], F32, isOutput=False)
    out_d = nc.declare_dram_parameter("out", [512, D], F32, isOutput=True)

    Exp = mybir.ActivationFunctionType.Exp

    with tile.TileContext(nc) as tc, ExitStack() as ctx:
        const_pool = ctx.enter_context(tc.tile_pool(name="const", bufs=1))
        aug_pool = ctx.enter_context(tc.tile_pool(name="aug", bufs=1))
        v_pool = ctx.enter_context(tc.tile_pool(name="vsb", bufs=1))
        at_pool = ctx.enter_context(tc.tile_pool(name="atsb", bufs=1))
        dram_pool = ctx.enter_context(tc.tile_pool(name="dram", bufs=1, space="DRAM"))

        ident = const_pool.tile([128, 128], F32, name="ident")
        make_identity(nc, ident[:])
        cs_sb = const_pool.tile([128, N], F32, name="cs_sb")
        nc.sync.dma_start(cs_sb[:], cs_d[:])
        cos64 = cs_sb[0:64, :]
        sin64 = cs_sb[64:128, :]
        gm_sb = const_pool.tile([128, 2], F32, name="gm_sb")
        nc.sync.dma_start(gm_sb[:], gm_d[:])

        # q/k augmented feature-major tiles: rows 0-63 head dims (rope heads
        # stored evens-then-odds), row 64/65 the bias augmentation.
        qaug, kaug = [], []
        for h in range(4):
            qa = aug_pool.tile([66, N], F32, name=f"qaug{h}")
            ka = aug_pool.tile([66, N], F32, name=f"kaug{h}")
            nc.sync.dma_start(ka[64:65, :], beta_d[h : h + 1, :])
            nc.sync.dma_start(ka[65:66, :], beta_d[8:9, :])
            nc.sync.dma_start(qa[64:65, :], beta_d[8:9, :])
            nc.sync.dma_start(qa[65:66, :], beta_d[4 + h : 5 + h, :])
            qaug.append(qa)
            kaug.append(ka)

        # V natural layout [k, dh] per head + ones column for free row sums.
        vsb = []
        for h in range(4):
            v_t = v_pool.tile([128, NT, 65], F32, name=f"v{h}")
            nc.gpsimd.memset(v_t[:, :, 64:65], 1.0)
            vsb.append(v_t)

        # A^T accumulation [features, n]: tile t holds heads 2t, 2t+1.
        atsb = [at_pool.tile([128, N], F32, name=f"at{t}") for t in range(2)]

        # ---------------- phase 1: projections -----------------------------
        with ExitStack() as p1:
            w_pool = p1.enter_context(tc.tile_pool(name="wsb", bufs=1))
            xt_pool = p1.enter_context(tc.tile_pool(name="xtp", bufs=3))
            qk_pspool = p1.enter_context(tc.tile_pool(name="qkps", bufs=4, space="PSUM"))
            v_pspool = p1.enter_context(tc.tile_pool(name="vps", bufs=4, space="PSUM"))
            rope_pool = p1.enter_context(tc.tile_pool(name="rope", bufs=2))

            wsb = []
            for kt in range(KT):
                w_t = w_pool.tile([128, 768], F32, name=f"w{kt}")
                nc.sync.dma_start(w_t[:], wqkv_d[kt * 128 : (kt + 1) * 128, :])
                wsb.append(w_t)

            for ch in range(NCH):
                cc = slice(ch * 512, (ch + 1) * 512)
                qk_ps = [qk_pspool.tile([128, 512], F32, tag="qkps", name=f"qk{ch}_{j}") for j in range(4)]
                v_ps = [v_pspool.tile([128, 256], F32, tag="vps", name=f"vp{ch}_{s}") for s in range(4)]
                for kt in range(KT):
                    x_t = xt_pool.tile([128, 512], F32, tag="xt", name=f"x{ch}_{kt}")
                    nc.sync.dma_start(x_t[:], xt_d[kt * 128 : (kt + 1) * 128, cc])
                    for j in range(4):
                        nc.tensor.matmul(
                            qk_ps[j][:],
                            wsb[kt][:, j * 128 : (j + 1) * 128],
                            x_t[:],
                            start=(kt == 0),
                            stop=(kt == KT - 1),
                        )
                    for s in range(4):
                        nc.tensor.matmul(
                            v_ps[s][:],
                            x_t[:, s * 128 : (s + 1) * 128],
                            wsb[kt][:, 512:768],
                            start=(kt == 0),
                            stop=(kt == KT - 1),
                        )
                # non-rope heads (0,1): plain copies (q already scaled on host)
                nc.scalar.copy(qaug[0][0:64, cc], qk_ps[0][0:64, :])
                nc.scalar.copy(qaug[1][0:64, cc], qk_ps[0][64:128, :])
                nc.scalar.copy(kaug[0][0:64, cc], qk_ps[2][0:64, :])
                nc.scalar.copy(kaug[1][0:64, cc], qk_ps[2][64:128, :])
                # rope heads (2,3): psum rows = [h2e|h3e|h2o|h3o]
                for ps, dst in ((qk_ps[1], qaug), (qk_ps[3], kaug)):
                    t1 = rope_pool.tile([64, 512], F32, tag="t1", name=f"t1_{ch}")
                    t2 = rope_pool.tile([64, 512], F32, tag="t2", name=f"t2_{ch}")
                    top = rope_pool.tile([64, 512], F32, tag="top", name=f"tp_{ch}")
                    bot = rope_pool.tile([64, 512], F32, tag="bot", name=f"bt_{ch}")
                    nc.vector.tensor_mul(t1[:], ps[0:64, :], cos64[:, cc])
                    nc.vector.tensor_mul(t2[:], ps[64:128, :], sin64[:, cc])
                    nc.vector.tensor_sub(top[:], t1[:], t2[:])
                    nc.vector.tensor_mul(t1[:], ps[0:64, :], sin64[:, cc])
                    nc.vector.tensor_mul(t2[:], ps[64:128, :], cos64[:, cc])
                    nc.vector.tensor_add(bot[:], t1[:], t2[:])
                    nc.gpsimd.tensor_copy(dst[2][0:32, cc], top[0:32, :])
                    nc.gpsimd.tensor_copy(dst[3][0:32, cc], top[32:64, :])
                    nc.gpsimd.tensor_copy(dst[2][32:64, cc], bot[0:32, :])
                    nc.gpsimd.tensor_copy(dst[3][32:64, cc], bot[32:64, :])
                for s in range(4):
                    for h in range(4):
                        nc.vector.tensor_copy(
                            vsb[h][:, ch * 4 + s, 0:64], v_ps[s][:, h * 64 : (h + 1) * 64]
                        )

        # ---------------- phase 2: attention --------------------------------
        with ExitStack() as p2:
            st_pool = p2.enter_context(tc.tile_pool(name="stps", bufs=4, space="PSUM"))
            av_pool = p2.enter_context(tc.tile_pool(name="avps", bufs=2, space="PSUM"))
            atp_pool = p2.enter_context(tc.tile_pool(name="atps", bufs=2, space="PSUM"))
            pt_pool = p2.enter_context(tc.tile_pool(name="ptsb", bufs=5))
            a_pool = p2.enter_context(tc.tile_pool(name="asb", bufs=4))

            for h in range(4):
                for p in range(NT // 2):  # pairs of query tiles
                    q0, q1 = 2 * p, 2 * p + 1
                    qc = slice(q0 * 128, q0 * 128 + 256)
                    av0 = av_pool.tile([128, 65], F32, tag="av", name=f"av0_{h}_{p}")
                    av1 = av_pool.tile([128, 65], F32, tag="av", name=f"av1_{h}_{p}")
                    for kb in range(q1 + 1):
                        st = st_pool.tile([128, 256], F32, tag="st", name=f"st{h}_{p}_{kb}")
                        nc.tensor.matmul(
                            st[:],
                            kaug[h][:, kb * 128 : (kb + 1) * 128],
                            qaug[h][:, qc],
                            start=True,
                            stop=True,
                        )
                        pt = pt_pool.tile([128, 256], F32, tag="pt", name=f"pt{h}_{p}_{kb}")
                        nc.scalar.activation(pt[:], st[:], Exp)
                        if kb == q0:
                            # diag for q-tile q0: keep k<=q in left half
                            nc.gpsimd.affine_select(
                                out=pt[:, 0:128],
                                in_=pt[:, 0:128],
                                compare_op=mybir.AluOpType.is_ge,
                                fill=0.0,
                                base=0,
                                pattern=[[1, 128]],
                                channel_multiplier=-1,
                            )
                        if kb == q1:
                            nc.gpsimd.affine_select(
                                out=pt[:, 128:256],
                                in_=pt[:, 128:256],
                                compare_op=mybir.AluOpType.is_ge,
                                fill=0.0,
                                base=0,
                                pattern=[[1, 128]],
                                channel_multiplier=-1,
                            )
                        if kb <= q0:
                            nc.tensor.matmul(
                                av0[:],
                                pt[:, 0:128],
                                vsb[h][:, kb, :],
                                start=(kb == 0),
                                stop=(kb == q0),
                                skip_group_check=True,
                            )
                        nc.tensor.matmul(
                            av1[:],
                            pt[:, 128:256],
                            vsb[h][:, kb, :],
                            start=(kb == 0),
                            stop=(kb == q1),
                            skip_group_check=True,
                        )
                    for qt, av in ((q0, av0), (q1, av1)):
                        rec = a_pool.tile([128, 1], F32, tag="rec", name=f"rc{h}_{qt}")
                        nc.vector.reciprocal(rec[:], av[:, 64:65])
                        a_sb = a_pool.tile([128, 64], F32, tag="asb", name=f"as{h}_{qt}")
                        nc.vector.tensor_scalar_mul(a_sb[:], av[:, 0:64], rec[:])
                        atp = atp_pool.tile([64, 128], F32, tag="atp", name=f"ap{h}_{qt}")
                        nc.tensor.transpose(atp[:], a_sb[:], ident[:])
                        t, row = h // 2, (h % 2) * 64
                        nc.scalar.copy(
                            atsb[t][row : row + 64, qt * 128 : (qt + 1) * 128], atp[:]
                        )

        # ---------------- phase 3: exchange + output projection -------------
        at_loc = dram_pool.tile([2048, 512], F32, name="at_loc")
        at_gath = dram_pool.tile([2048, 512], F32, name="at_gath")

        with ExitStack() as p3:
            stage_pool = p3.enter_context(tc.tile_pool(name="stg", bufs=2))
            atg_pool = p3.enter_context(tc.tile_pool(name="atg", bufs=1))
            wo_pool = p3.enter_context(tc.tile_pool(name="wop", bufs=2))
            op_pspool = p3.enter_context(tc.tile_pool(name="opps", bufs=8, space="PSUM"))
            osb_pool = p3.enter_context(tc.tile_pool(name="osb", bufs=4))

            # shard i of at_loc = own A^T column-chunk (i%4), scaled by the
            # batch mask gm[i//4] (1 for own batch, 0 otherwise).
            for b in range(2):
                for t in range(2):
                    stg = stage_pool.tile([128, N], F32, tag="stg", name=f"sg{b}_{t}")
                    nc.scalar.activation(
                        stg[:],
                        atsb[t][:],
                        mybir.ActivationFunctionType.Copy,
                        bias=0.0,
                        scale=gm_sb[:, b : b + 1],
                    )
                    for j in range(4):
                        nc.sync.dma_start(
                            at_loc[b * 1024 + j * 256 + t * 128 : b * 1024 + j * 256 + (t + 1) * 128, :],
                            stg[:, j * 512 : (j + 1) * 512],
                        )
            nc.gpsimd.collective_compute(
                "AllToAll",
                mybir.AluOpType.bypass,
                replica_groups=[[0, 1, 2, 3, 4, 5, 6, 7]],
                ins=[at_loc.opt()],
                outs=[at_gath.opt()],
            )
            # fold the two batch halves (one is zero) -> full [1024, 512] A^T
            atg_sb = []
            for kt in range(KT):
                h0 = atg_pool.tile([128, 512], F32, tag=f"g{kt}", name=f"g0_{kt}")
                h1 = atg_pool.tile([128, 512], F32, tag=f"h{kt}", name=f"g1_{kt}")
                nc.sync.dma_start(h0[:], at_gath[kt * 128 : (kt + 1) * 128, :])
                nc.sync.dma_start(h1[:], at_gath[1024 + kt * 128 : 1024 + (kt + 1) * 128, :])
                nc.vector.tensor_add(h0[:], h0[:], h1[:])
                atg_sb.append(h0)

            op_ps = [
                [op_pspool.tile([128, 512], F32, tag="op", name=f"o{qt}_{hf}") for hf in range(2)]
                for qt in range(4)
            ]
            for kt in range(KT):
                wo_t = wo_pool.tile([128, D], F32, tag="wo", name=f"wo{kt}")
                nc.sync.dma_start(wo_t[:], wo_d[kt * 128 : (kt + 1) * 128, :])
                for qt in range(4):
                    for hf in range(2):
                        nc.tensor.matmul(
                            op_ps[qt][hf][:],
                            atg_sb[kt][:, qt * 128 : (qt + 1) * 128],
                            wo_t[:, hf * 512 : (hf + 1) * 512],
                            start=(kt == 0),
                            stop=(kt == KT - 1),
                        )
            for qt in range(4):
                for hf in range(2):
                    o_sb = osb_pool.tile([128, 512], F32, tag="osb", name=f"ob{qt}_{hf}")
                    nc.vector.tensor_copy(o_sb[:], op_ps[qt][hf][:])
                    nc.sync.dma_start(
                        out_d[qt * 128 : (qt + 1) * 128, hf * 512 : (hf + 1) * 512],
                        o_sb[:],
                    )

    nc.compile()
    return nc


def get_nc():
    if "nc" not in _CACHE:
        _CACHE["nc"] = _build()
    return _CACHE["nc"]


def prep_shards(x, w_qkv, w_o, log_xi):
    x = np.asarray(x, np.float32)
    w_qkv = np.asarray(w_qkv, np.float32)
    w_o = np.ascontiguousarray(np.asarray(w_o, np.float32))
    log_xi = np.asarray(log_xi)

    pos = np.arange(N, dtype=np.float64)
    inv = ROPE_BASE ** (-(np.arange(0, DH, 2, dtype=np.float64) / DH))
    th = np.outer(pos, inv)  # (N, 32)
    cosT = np.cos(th).T.astype(np.float32)
    sinT = np.sin(th).T.astype(np.float32)
    cs = np.ascontiguousarray(np.concatenate([cosT, cosT, sinT, sinT], 0))  # (128, N)

    xi = np.exp(log_xi.astype(np.float64))
    slope = E_BITS / xi  # (H,)

    perm_e = np.arange(0, DH, 2)
    perm_o = np.arange(1, DH, 2)

    def wcols(h, blk):
        return w_qkv[:, blk * D + h * DH : blk * D + (h + 1) * DH]

    in_maps = []
    for r in range(8):
        g, c = divmod(r, 4)
        hs = [4 * c + i for i in range(4)]
        xt = np.ascontiguousarray(x[g].T)

        q01 = [wcols(hs[0], 0) * 0.125, wcols(hs[1], 0) * 0.125]
        h2q, h3q = wcols(hs[2], 0) * 0.125, wcols(hs[3], 0) * 0.125
        ropeq = np.concatenate(
            [h2q[:, perm_e], h3q[:, perm_e], h2q[:, perm_o], h3q[:, perm_o]], 1
        )
        k01 = [wcols(hs[0], 1), wcols(hs[1], 1)]
        h2k, h3k = wcols(hs[2], 1), wcols(hs[3], 1)
        ropek = np.concatenate(
            [h2k[:, perm_e], h3k[:, perm_e], h2k[:, perm_o], h3k[:, perm_o]], 1
        )
        vb = [wcols(h, 2) for h in hs]
        wqkv_loc = np.ascontiguousarray(
            np.concatenate(q01 + [ropeq] + k01 + [ropek] + vb, 1), np.float32
        )

        beta = np.zeros((10, N), np.float32)
        beta[8] = 1.0
        for i, h in enumerate(hs):
            beta[i] = (slope[h] * pos).astype(np.float32)
            beta[4 + i] = (-slope[h] * pos).astype(np.float32)

        gm = np.zeros((128, 2), np.float32)
        gm[:, g] = 1.0

        in_maps.append(
            {"xt": xt, "wqkv": wqkv_loc, "wo": w_o, "cs": cs, "beta": beta, "gm": gm}
        )
    return in_maps


def run(in_maps, trace=False):
    nc = get_nc()
    return run_bass_kernel_spmd(nc, in_maps, list(range(8)), trace=trace)


def kernel(x, w_qkv, w_o, log_xi):
    in_maps = prep_shards(x, w_qkv, w_o, log_xi)
    res = run(in_maps)
    out = np.empty((B, N, D), np.float32)
    for r in range(8):
        g, c = divmod(r, 4)
        out[g, c * 512 : (c + 1) * 512, :] = res.results[r]["out"]
    return out


# revision 8
# speedup vs baseline: 1.9506x; 1.9506x over previous
"""Trainium2 Bass kernel for nn_EulerCEMinimal (mixed-head causal attention).

Reference model (B=2, N=2048, D=1024, H=16, dh=64):
  qkv = x @ w_qkv;  heads round-robin {nope, alibi, rope, xpos}
  rope/xpos heads: interleaved even/odd RoPE rotation of q,k
  alibi/xpos heads: additive decay bias -|q-k|/xi
  causal softmax;  out = (attn @ v) @ w_o

Sharding (8 cores): core r = (batch g=r//4) x (head-group c=r%4, heads 4c..4c+3).
Each core projects q/k transposed (feature-major). Scores are computed
transposed S^T[k,q] with a K=65 fp32r contraction whose augmented row carries
-beta[q] (a per-row constant whose rounding cancels exactly in softmax); the
per-key decay bias +beta[k] is applied in full fp32 as the per-partition bias
of the exp activation. Causality comes from computing only k<=q blocks plus an
affine_select on the diagonal. Softmax normalization is deferred (row sums
come free from a ones-column appended to V) and folded into the attention
output. Head outputs are exchanged with an 8-core AllToAll (cross-batch shards
zeroed via a data mask, folded by an add on the receiver) so each core
computes its own 512-row slice of the output projection. All rank dependence
lives in the input data; the SPMD program is identical on every core.
"""

import sys
import types

sys.path.insert(0, "/opt/trn_rl_repo")

# Register the NTFF profile hook that the stub antenv package lacks (used only
# when tracing is requested, e.g. BASS_TRACE=1 or trace=True).
if "antenv.axon_hooks" not in sys.modules:
    _hooks = types.ModuleType("antenv.axon_hooks")
    _hooks._hook = None
    _hooks.set_axon_ntff_profile_hook = lambda h: setattr(_hooks, "_hook", h)
    _hooks.get_axon_ntff_profile_hook = lambda: _hooks._hook
    sys.modules["antenv.axon_hooks"] = _hooks
    try:
        from trn_agent_boot.trn_boot import _ntff_profile_via_ctypes

        _hooks.set_axon_ntff_profile_hook(
            _ntff_profile_via_ctypes("/opt/axon/libaxon_pjrt.so")
        )
    except Exception:
        pass

from contextlib import ExitStack

import numpy as np

import concourse.bass as bass
import concourse.tile as tile
from concourse import bacc, mybir
from concourse.bass_utils import run_bass_kernel_spmd
from concourse.masks import make_identity

F32 = mybir.dt.float32
F32R = mybir.dt.float32r
B, N, D, H, DH = 2, 2048, 1024, 16, 64
NT = N // 128  # 16 query/key tiles
NCH = N // 512  # 4 projection column chunks
KT = D // 128  # 8 contraction tiles
ROPE_BASE = 10000.0
HEAD_TYPES = np.arange(H) % 4
PI_BITS = (HEAD_TYPES >> 1) & 1  # rotate (rope/xpos)
E_BITS = HEAD_TYPES & 1  # decay (alibi/xpos)

_CACHE = {}


def round_f32r(a):
    """Round fp32 array to fp32r (11-bit mantissa, round-to-nearest-even)."""
    u = np.ascontiguousarray(a, np.float32).view(np.uint32)
    hi = u >> 12
    low = u & 0xFFF
    add = ((low > 0x800) | ((low == 0x800) & ((hi & 1) == 1))).astype(np.uint32)
    return ((hi + add) << 12).view(np.float32)


def _build():
    nc = bacc.Bacc(None, target_bir_lowering=False, debug=False, num_devices=8)
    xt_d = nc.declare_dram_parameter("xt", [D, N], F32R, isOutput=False)
    wqkv_d = nc.declare_dram_parameter("wqkv", [D, 768], F32R, isOutput=False)
    wo_d = nc.declare_dram_parameter("wo", [D, D], F32R, isOutput=False)
    cs_d = nc.declare_dram_parameter("cs", [128, N], F32, isOutput=False)
    beta_d = nc.declare_dram_parameter("beta", [6, N], F32R, isOutput=False)
    betac_d = nc.declare_dram_parameter("betac", [128, 64], F32, isOutput=False)
    gm_d = nc.declare_dram_parameter("gm", [128, 2], F32, isOutput=False)
    out_d = nc.declare_dram_parameter("out", [512, D], F32, isOutput=True)

    Exp = mybir.ActivationFunctionType.Exp

    with tile.TileContext(nc) as tc, ExitStack() as ctx:
        const_pool = ctx.enter_context(tc.tile_pool(name="const", bufs=1))
        aug_pool = ctx.enter_context(tc.tile_pool(name="aug", bufs=1))
        v_pool = ctx.enter_context(tc.tile_pool(name="vsb", bufs=1))
        at_pool = ctx.enter_context(tc.tile_pool(name="atsb", bufs=1))
        dram_pool = ctx.enter_context(tc.tile_pool(name="dram", bufs=1, space="DRAM"))

        ident = const_pool.tile([128, 128], F32, name="ident")
        make_identity(nc, ident[:])
        cs_sb = const_pool.tile([128, N], F32, name="cs_sb")
        nc.sync.dma_start(cs_sb[:], cs_d[:])
        cos64 = cs_sb[0:64, :]
        sin64 = cs_sb[64:128, :]
        gm_sb = const_pool.tile([128, 2], F32, name="gm_sb")
        nc.sync.dma_start(gm_sb[:], gm_d[:])
        betac_sb = const_pool.tile([128, 64], F32, name="betac_sb")
        nc.sync.dma_start(betac_sb[:], betac_d[:])

        # q/k feature-major tiles (fp32r): rows 0-63 head dims (rope heads
        # stored evens-then-odds); row 64 = ones (k) / -beta[q] (q).
        qaug, kaug = [], []
        for h in range(4):
            qa = aug_pool.tile([65, N], F32R, name=f"qaug{h}")
            ka = aug_pool.tile([65, N], F32R, name=f"kaug{h}")
            nc.sync.dma_start(ka[64:65, :], beta_d[4:5, :])  # ones
            nc.sync.dma_start(qa[64:65, :], beta_d[h : h + 1, :])  # -beta[q]
            qaug.append(qa)
            kaug.append(ka)

        # V natural layout [k, dh] per head + ones column for free row sums.
        ones_t = const_pool.tile([128, NT, 2], F32, name="ones_t")
        nc.gpsimd.memset(ones_t[:, :, 0:1], 1.0)
        nc.gpsimd.memset(ones_t[:, :, 1:2], 0.0)
        vsb = []
        for h in range(4):
            v_t = v_pool.tile([128, NT, 66], F32R, name=f"v{h}")
            nc.vector.tensor_copy(v_t[:, :, 64:66], ones_t[:])
            vsb.append(v_t)

        # A^T accumulation [features, n]: tile t holds heads 2t, 2t+1.
        atsb = [at_pool.tile([128, N], F32, name=f"at{t}") for t in range(2)]

        # ---------------- phase 1: projections -----------------------------
        with ExitStack() as p1:
            w_pool = p1.enter_context(tc.tile_pool(name="wsb", bufs=1))
            xt_pool = p1.enter_context(tc.tile_pool(name="xtp", bufs=3))
            qk_pspool = p1.enter_context(tc.tile_pool(name="qkps", bufs=4, space="PSUM"))
            v_pspool = p1.enter_context(tc.tile_pool(name="vps", bufs=4, space="PSUM"))
            rope_pool = p1.enter_context(tc.tile_pool(name="rope", bufs=2))

            wsb = []
            for kt in range(KT):
                w_t = w_pool.tile([128, 768], F32R, name=f"w{kt}")
                nc.sync.dma_start(w_t[:], wqkv_d[kt * 128 : (kt + 1) * 128, :])
                wsb.append(w_t)

            for ch in range(NCH):
                cc = slice(ch * 512, (ch + 1) * 512)
                qk_ps = [qk_pspool.tile([128, 512], F32, tag="qkps", name=f"qk{ch}_{j}") for j in range(4)]
                v_ps = [v_pspool.tile([128, 256], F32, tag="vps", name=f"vp{ch}_{s}") for s in range(4)]
                for kt in range(KT):
                    x_t = xt_pool.tile([128, 512], F32R, tag="xt", name=f"x{ch}_{kt}")
                    nc.sync.dma_start(x_t[:], xt_d[kt * 128 : (kt + 1) * 128, cc])
                    for j in range(4):
                        nc.tensor.matmul(
                            qk_ps[j][:],
                            wsb[kt][:, j * 128 : (j + 1) * 128],
                            x_t[:],
                            start=(kt == 0),
                            stop=(kt == KT - 1),
                        )
                    for s in range(4):
                        nc.tensor.matmul(
                            v_ps[s][:],
                            x_t[:, s * 128 : (s + 1) * 128],
                            wsb[kt][:, 512:768],
                            start=(kt == 0),
                            stop=(kt == KT - 1),
                        )
                # non-rope heads (0,1): plain copies (q already scaled on host)
                nc.scalar.copy(qaug[0][0:64, cc], qk_ps[0][0:64, :])
                nc.scalar.copy(qaug[1][0:64, cc], qk_ps[0][64:128, :])
                nc.scalar.copy(kaug[0][0:64, cc], qk_ps[2][0:64, :])
                nc.scalar.copy(kaug[1][0:64, cc], qk_ps[2][64:128, :])
                # rope heads (2,3): psum rows = [h2e|h3e|h2o|h3o]
                for ps, dst in ((qk_ps[1], qaug), (qk_ps[3], kaug)):
                    t1 = rope_pool.tile([64, 512], F32, tag="t1", name=f"t1_{ch}")
                    t2 = rope_pool.tile([64, 512], F32, tag="t2", name=f"t2_{ch}")
                    top = rope_pool.tile([64, 512], F32, tag="top", name=f"tp_{ch}")
                    bot = rope_pool.tile([64, 512], F32, tag="bot", name=f"bt_{ch}")
                    nc.vector.tensor_mul(t1[:], ps[0:64, :], cos64[:, cc])
                    nc.vector.tensor_mul(t2[:], ps[64:128, :], sin64[:, cc])
                    nc.vector.tensor_sub(top[:], t1[:], t2[:])
                    nc.vector.tensor_mul(t1[:], ps[0:64, :], sin64[:, cc])
                    nc.vector.tensor_mul(t2[:], ps[64:128, :], cos64[:, cc])
                    nc.vector.tensor_add(bot[:], t1[:], t2[:])
                    nc.vector.tensor_copy(dst[2][0:32, cc], top[0:32, :])
                    nc.vector.tensor_copy(dst[3][0:32, cc], top[32:64, :])
                    nc.vector.tensor_copy(dst[2][32:64, cc], bot[0:32, :])
                    nc.vector.tensor_copy(dst[3][32:64, cc], bot[32:64, :])
                for s in range(4):
                    for h in range(4):
                        nc.vector.tensor_copy(
                            vsb[h][:, ch * 4 + s, 0:64], v_ps[s][:, h * 64 : (h + 1) * 64]
                        )

        # ---------------- phase 2: attention --------------------------------
        with ExitStack() as p2:
            st_pool = p2.enter_context(tc.tile_pool(name="stps", bufs=4, space="PSUM"))
            av_pool = p2.enter_context(tc.tile_pool(name="avps", bufs=2, space="PSUM"))
            atp_pool = p2.enter_context(tc.tile_pool(name="atps", bufs=2, space="PSUM"))
            pt_pool = p2.enter_context(tc.tile_pool(name="ptsb", bufs=5))
            a_pool = p2.enter_context(tc.tile_pool(name="asb", bufs=4))

            for h in range(4):
                for p in range(NT // 2):  # pairs of query tiles
                    q0, q1 = 2 * p, 2 * p + 1
                    qc = slice(q0 * 128, q0 * 128 + 256)
                    av0 = av_pool.tile([128, 66], F32, tag="av", name=f"av0_{h}_{p}")
                    av1 = av_pool.tile([128, 66], F32, tag="av", name=f"av1_{h}_{p}")
                    for kb in range(q1 + 1):
                        st = st_pool.tile([128, 256], F32, tag="st", name=f"st{h}_{p}_{kb}")
                        nc.tensor.matmul(
                            st[:],
                            kaug[h][:, kb * 128 : (kb + 1) * 128],
                            qaug[h][:, qc],
                            start=True,
                            stop=True,
                        )
                        pt = pt_pool.tile([128, 256], F32R, tag="pt", name=f"pt{h}_{p}_{kb}")
                        nc.scalar.activation(
                            pt[:], st[:], Exp, bias=betac_sb[:, h * 16 + kb : h * 16 + kb + 1]
                        )
                        if kb == q0:
                            # diag for q-tile q0: keep k<=q in left half
                            nc.gpsimd.affine_select(
                                out=pt[:, 0:128],
                                in_=pt[:, 0:128],
                                compare_op=mybir.AluOpType.is_ge,
                                fill=0.0,
                                base=0,
                                pattern=[[1, 128]],
                                channel_multiplier=-1,
                            )
                        if kb == q1:
                            nc.gpsimd.affine_select(
                                out=pt[:, 128:256],
                                in_=pt[:, 128:256],
                                compare_op=mybir.AluOpType.is_ge,
                                fill=0.0,
                                base=0,
                                pattern=[[1, 128]],
                                channel_multiplier=-1,
                            )
                        if kb <= q0:
                            nc.tensor.matmul(
                                av0[:],
                                pt[:, 0:128],
                                vsb[h][:, kb, :],
                                start=(kb == 0),
                                stop=(kb == q0),
                                skip_group_check=True,
                            )
                        nc.tensor.matmul(
                            av1[:],
                            pt[:, 128:256],
                            vsb[h][:, kb, :],
                            start=(kb == 0),
                            stop=(kb == q1),
                            skip_group_check=True,
                        )
                    for qt, av in ((q0, av0), (q1, av1)):
                        rec = a_pool.tile([128, 1], F32, tag="rec", name=f"rc{h}_{qt}")
                        nc.vector.reciprocal(rec[:], av[:, 64:65])
                        a_sb = a_pool.tile([128, 64], F32, tag="asb", name=f"as{h}_{qt}")
                        nc.vector.tensor_scalar_mul(a_sb[:], av[:, 0:64], rec[:])
                        atp = atp_pool.tile([64, 128], F32, tag="atp", name=f"ap{h}_{qt}")
                        nc.tensor.transpose(atp[:], a_sb[:], ident[:])
                        t, row = h // 2, (h % 2) * 64
                        nc.vector.tensor_copy(
                            atsb[t][row : row + 64, qt * 128 : (qt + 1) * 128], atp[:]
                        )

        # ---------------- phase 3: exchange + output projection -------------
        at_loc = dram_pool.tile([2048, 512], F32, name="at_loc")
        at_gath = dram_pool.tile([2048, 512], F32, name="at_gath")

        with ExitStack() as p3:
            stage_pool = p3.enter_context(tc.tile_pool(name="stg", bufs=2))
            atg_pool = p3.enter_context(tc.tile_pool(name="atg", bufs=1))
            fold_pool = p3.enter_context(tc.tile_pool(name="fold", bufs=4))
            wo_pool = p3.enter_context(tc.tile_pool(name="wop", bufs=2))
            op_pspool = p3.enter_context(tc.tile_pool(name="opps", bufs=8, space="PSUM"))
            osb_pool = p3.enter_context(tc.tile_pool(name="osb", bufs=4))

            # shard i of at_loc = own A^T column-chunk (i%4), scaled by the
            # batch mask gm[i//4] (1 for own batch, 0 otherwise).
            for b in range(2):
                for t in range(2):
                    stg = stage_pool.tile([128, N], F32, tag="stg", name=f"sg{b}_{t}")
                    nc.scalar.activation(
                        stg[:],
                        atsb[t][:],
                        mybir.ActivationFunctionType.Copy,
                        bias=0.0,
                        scale=gm_sb[:, b : b + 1],
                    )
                    for j in range(4):
                        nc.sync.dma_start(
                            at_loc[b * 1024 + j * 256 + t * 128 : b * 1024 + j * 256 + (t + 1) * 128, :],
                            stg[:, j * 512 : (j + 1) * 512],
                        )
            nc.gpsimd.collective_compute(
                "AllToAll",
                mybir.AluOpType.bypass,
                replica_groups=[[0, 1, 2, 3, 4, 5, 6, 7]],
                ins=[at_loc.opt()],
                outs=[at_gath.opt()],
            )
            # fold the two batch halves (one is zero) -> full [1024, 512] A^T
            atg_sb = []
            for kt in range(KT):
                h0 = fold_pool.tile([128, 512], F32, tag="h0", name=f"g0_{kt}")
                h1 = fold_pool.tile([128, 512], F32, tag="h1", name=f"g1_{kt}")
                nc.sync.dma_start(h0[:], at_gath[kt * 128 : (kt + 1) * 128, :])
                nc.sync.dma_start(h1[:], at_gath[1024 + kt * 128 : 1024 + (kt + 1) * 128, :])
                fr = atg_pool.tile([128, 512], F32R, tag=f"fr{kt}", name=f"fr{kt}")
                nc.vector.tensor_add(fr[:], h0[:], h1[:])
                atg_sb.append(fr)

            op_ps = [
                [op_pspool.tile([128, 512], F32, tag="op", name=f"o{qt}_{hf}") for hf in range(2)]
                for qt in range(4)
            ]
            for kt in range(KT):
                wo_t = wo_pool.tile([128, D], F32R, tag="wo", name=f"wo{kt}")
                nc.sync.dma_start(wo_t[:], wo_d[kt * 128 : (kt + 1) * 128, :])
                for qt in range(4):
                    for hf in range(2):
                        nc.tensor.matmul(
                            op_ps[qt][hf][:],
                            atg_sb[kt][:, qt * 128 : (qt + 1) * 128],
                            wo_t[:, hf * 512 : (hf + 1) * 512],
                            start=(kt == 0),
                            stop=(kt == KT - 1),
                        )
            for qt in range(4):
                for hf in range(2):
                    o_sb = osb_pool.tile([128, 512], F32, tag="osb", name=f"ob{qt}_{hf}")
                    nc.vector.tensor_copy(o_sb[:], op_ps[qt][hf][:])
                    nc.sync.dma_start(
                        out_d[qt * 128 : (qt + 1) * 128, hf * 512 : (hf + 1) * 512],
                        o_sb[:],
                    )

    nc.compile()
    return nc


def get_nc():
    if "nc" not in _CACHE:
        _CACHE["nc"] = _build()
    return _CACHE["nc"]


def prep_shards(x, w_qkv, w_o, log_xi):
    x = np.asarray(x, np.float32)
    w_qkv = np.asarray(w_qkv, np.float32)
    w_o = round_f32r(np.asarray(w_o, np.float32))
    log_xi = np.asarray(log_xi)

    pos = np.arange(N, dtype=np.float64)
    inv = ROPE_BASE ** (-(np.arange(0, DH, 2, dtype=np.float64) / DH))
    th = np.outer(pos, inv)  # (N, 32)
    cosT = np.cos(th).T.astype(np.float32)
    sinT = np.sin(th).T.astype(np.float32)
    cs = np.ascontiguousarray(np.concatenate([cosT, cosT, sinT, sinT], 0))  # (128, N)

    xi = np.exp(log_xi.astype(np.float64))
    slope = E_BITS / xi  # (H,)

    perm_e = np.arange(0, DH, 2)
    perm_o = np.arange(1, DH, 2)

    def wcols(h, blk):
        return w_qkv[:, blk * D + h * DH : blk * D + (h + 1) * DH]

    in_maps = []
    for r in range(8):
        g, c = divmod(r, 4)
        hs = [4 * c + i for i in range(4)]
        xt = round_f32r(x[g].T)

        q01 = [wcols(hs[0], 0) * 0.125, wcols(hs[1], 0) * 0.125]
        h2q, h3q = wcols(hs[2], 0) * 0.125, wcols(hs[3], 0) * 0.125
        ropeq = np.concatenate(
            [h2q[:, perm_e], h3q[:, perm_e], h2q[:, perm_o], h3q[:, perm_o]], 1
        )
        k01 = [wcols(hs[0], 1), wcols(hs[1], 1)]
        h2k, h3k = wcols(hs[2], 1), wcols(hs[3], 1)
        ropek = np.concatenate(
            [h2k[:, perm_e], h3k[:, perm_e], h2k[:, perm_o], h3k[:, perm_o]], 1
        )
        vb = [wcols(h, 2) for h in hs]
        wqkv_loc = round_f32r(np.concatenate(q01 + [ropeq] + k01 + [ropek] + vb, 1))

        # beta rows: 0-3 = -beta[q] per head (fp32r-rounded; exact row
        # constant cancels in softmax), 4 = ones.
        beta = np.zeros((6, N), np.float32)
        for i, h in enumerate(hs):
            beta[i] = -(slope[h] * pos).astype(np.float32)
        beta[4] = 1.0
        beta = round_f32r(beta)

        # per-key decay bias columns, full fp32: betac[p, h*16+kb] =
        # slope_h * (kb*128 + p)
        betac = np.zeros((128, 64), np.float32)
        prow = np.arange(128, dtype=np.float64)
        for i, h in enumerate(hs):
            for kb in range(16):
                betac[:, i * 16 + kb] = (slope[h] * (kb * 128 + prow)).astype(np.float32)

        gm = np.zeros((128, 2), np.float32)
        gm[:, g] = 1.0

        in_maps.append(
            {
                "xt": xt,
                "wqkv": wqkv_loc,
                "wo": w_o,
                "cs": cs,
                "beta": beta,
                "betac": betac,
                "gm": gm,
            }
        )
    return in_maps


def run(in_maps, trace=False):
    nc = get_nc()
    return run_bass_kernel_spmd(nc, in_maps, list(range(8)), trace=trace)


def kernel(x, w_qkv, w_o, log_xi):
    in_maps = prep_shards(x, w_qkv, w_o, log_xi)
    res = run(in_maps)
    out = np.empty((B, N, D), np.float32)
    for r in range(8):
        g, c = divmod(r, 4)
        out[g, c * 512 : (c + 1) * 512, :] = res.results[r]["out"]
    return out
